# revision 17
# baseline (speedup 1.0000x reference)
"""nn_MultiHeadAttention on 8 Trainium2 NeuronCores (Bass/Tile).

Reference computation (torch nn.MultiheadAttention, eval, causal):
    q = x @ wq + bq ; k = x @ wk + bk ; v = x @ wv + bv   (16 heads of 64)
    out = softmax(causal(q k^T / 8)) @ v ;  y = out @ wo + bo

Sharding (8 cores): head-parallel attention (2 heads/core, all batches),
then token-parallel output projection after per-batch AllToAlls.

Per-core pipeline:
  0. cast its token strips of x to bf16, per-batch AllGather -> full bf16 x.
  1. x^T via xbar transpose-DMA; project Q^T/K^T ([hd,tok] bf16) and V
     ([tok,hd] with a ones column -> the PV matmul also yields the softmax
     denominators).
  2. Causal attention in the S^T layout [k-tok, q-tok]: per (q-span, k-tile)
     both heads' scores land in one 2-bank PSUM pair (disjoint PE row
     groups), one exp on ACT per pair (scale=1/8 folded in; no
     max-subtraction needed: |s/8| < ~7), diagonal tiles compute only live
     columns plus one [128,128] bf16 boundary mask applied after exp.
     exp->PV is software-pipelined with a small lookahead.
  3. Normalize O^T by 1/denom (partition_broadcast + reciprocal), per-batch
     AllToAll, then the out-projection for this core's token strip of that
     batch (bias row via a K=1 matmul with c = bv@wo + bo, exact because
     softmax rows sum to 1).
Host: scatters the 8 [B, 256, D] chunks back into [B, T, D].
"""

import sys

if "/opt/trn_rl_repo" not in sys.path:
    sys.path.insert(0, "/opt/trn_rl_repo")

import numpy as np

import concourse.bass as bass  # noqa: F401
import concourse.mybir as mybir
import concourse.tile as tile
from concourse import bacc
from concourse import bass_utils

B, T, D, H, HD = 4, 2048, 1024, 16, 64
NCORES = 8
HPC = H // NCORES          # heads per core (2)
DHC = HPC * HD             # head dims per core (128)
STRIP = T // NCORES        # 256 tokens/batch contributed to AG & owned in y
SPAN = 512                 # q span (PSUM bank = 512 fp32)
KT = 128                   # k tile
NSPAN = T // SPAN          # 4
DCH = D // 128             # 8 contraction chunks
F32 = mybir.dt.float32
BF16 = mybir.dt.bfloat16
AF = mybir.ActivationFunctionType
RG = [list(range(NCORES))]
LOOKAHEAD = 4              # exp->PV software pipeline depth


def _emit(nc, dram, const, sb, ps, dbg, xs, wq, wk, wv, wo, bq, bk, bv, bo, y,
          *, debug, no_cc):
    # ---- constants / weights ----
    wq_sb = const.tile([128, DCH, DHC], BF16, tag="wq_sb", name="wq_sb")
    wk_sb = const.tile([128, DCH, DHC], BF16, tag="wk_sb", name="wk_sb")
    wv_sb = const.tile([128, DCH, DHC], BF16, tag="wv_sb", name="wv_sb")
    wo_sb = const.tile([128, DCH, D], BF16, tag="wo_sb", name="wo_sb")
    nc.gpsimd.dma_start(out=wq_sb[:], in_=wq.rearrange("(c p) m -> p c m", p=128))
    nc.gpsimd.dma_start(out=wk_sb[:], in_=wk.rearrange("(c p) m -> p c m", p=128))
    nc.gpsimd.dma_start(out=wv_sb[:], in_=wv.rearrange("(c p) m -> p c m", p=128))
    nc.gpsimd.dma_start(out=wo_sb[:], in_=wo.rearrange("(c p) m -> p c m", p=128))
    bq_sb = const.tile([128, 1], F32, tag="bq_sb", name="bq_sb")
    bk_sb = const.tile([128, 1], F32, tag="bk_sb", name="bk_sb")
    nc.sync.dma_start(out=bq_sb[:], in_=bq.rearrange("(p m) -> p m", m=1))
    nc.sync.dma_start(out=bk_sb[:], in_=bk.rearrange("(p m) -> p m", m=1))
    bv_sb = const.tile([128, DCH], BF16, tag="bv_sb", name="bv_sb")
    nc.gpsimd.dma_start(out=bv_sb[:], in_=bv.rearrange("(c p) -> p c", p=128))
    bo_sb = const.tile([1, D], F32, tag="bo_sb", name="bo_sb")
    nc.sync.dma_start(out=bo_sb[:], in_=bo.rearrange("(o m) -> o m", o=1))

    ones_row = const.tile([1, 128], BF16, tag="ones_row", name="ones_row")
    nc.vector.memset(ones_row[:], 1.0)
    # boundary mask (multiplicative, post-exp): keep iff qoff >= p,
    # duplicated for the two head halves of an et pair tile.
    mask2 = const.tile([128, 2, KT], BF16, tag="mask2", name="mask2")
    nc.gpsimd.memset(mask2[:], 1.0)
    for u in range(2):
        nc.gpsimd.affine_select(
            out=mask2[:, u, :], in_=mask2[:, u, :],
            pattern=[[1, KT]], compare_op=mybir.AluOpType.is_ge,
            fill=0.0, base=0, channel_multiplier=-1)

    ident = const.tile([128, 128], BF16, tag="ident", name="ident")
    from concourse.masks import make_identity
    make_identity(nc, ident[:])

    # c = bv @ wo + bo (exact bias row for y)
    c_sb = const.tile([1, D], BF16, tag="c_sb", name="c_sb")
    for s in range(D // SPAN):
        cps = ps.tile([128, SPAN], F32, tag="pps", bufs=2, name=f"cps{s}")
        for ch in range(DCH):
            nc.tensor.matmul(cps[0:1, :], bv_sb[:, ch:ch + 1],
                             wo_sb[:, ch, s * SPAN:(s + 1) * SPAN],
                             start=(ch == 0), stop=(ch == DCH - 1))
        nc.vector.tensor_add(out=c_sb[0:1, s * SPAN:(s + 1) * SPAN],
                             in0=cps[0:1, :],
                             in1=bo_sb[0:1, s * SPAN:(s + 1) * SPAN])

    # ---- bf16 cast, PE-transpose own strip, AllGather of x^T (per batch) ----
    xg = []
    for b in range(B):
        xst_sb = sb.tile([128, 2, D], BF16, tag="xst", bufs=1, name=f"xst{b}")
        nc.gpsimd.dma_start(out=xst_sb[:],
                            in_=xs[b].rearrange("(g p) d -> p g d", p=128))
        xts = sb.tile([128, DCH, STRIP], BF16, tag="xts", bufs=2,
                      name=f"xts{b}")
        for g in range(2):
            for ch in range(DCH):
                tp = ps.tile([128, 128], BF16, tag="pps", bufs=2,
                             name=f"tp{b}_{g}_{ch}")
                nc.tensor.transpose(tp[:], xst_sb[:, g, ch * 128:(ch + 1) * 128],
                                    ident[:])
                nc.vector.tensor_copy(out=xts[:, ch, g * 128:(g + 1) * 128],
                                      in_=tp[:])
        xtd = dram.tile([D, STRIP], BF16, name=f"xtd{b}")
        nc.sync.dma_start(out=xtd.opt().rearrange("(c p) m -> p c m", p=128),
                          in_=xts[:])
        xgb = dram.tile([NCORES, D, STRIP], BF16, name=f"xg{b}",
                        addr_space="Local" if no_cc else "Shared")
        if no_cc:
            for _r in range(NCORES):
                nc.gpsimd.dma_start(out=xgb[_r], in_=xtd[:])
        else:
            nc.gpsimd.collective_compute(
                "AllGather", mybir.AluOpType.bypass, replica_groups=RG,
                ins=[xtd.opt()], outs=[xgb.opt()])
        xg.append(xgb)

    st = {}  # per-batch tiles

    def load_xT(b):
        # [128, ch, g, m]: d = 128*ch + p, token = 256*g + m
        xT = sb.tile([128, DCH, NCORES, STRIP], BF16, tag="xT", bufs=1,
                     name=f"xT{b}")
        for g in range(NCORES):
            nc.sync.dma_start(
                out=xT[:, :, g, :],
                in_=xg[b][g].rearrange("(c p) m -> p c m", p=128))
        st[("xT", b)] = xT

    def proj_qk(b, which):
        xT = st[("xT", b)]
        w_sb, b_sb = (wq_sb, bq_sb) if which == "q" else (wk_sb, bk_sb)
        out = sb.tile([128, T], BF16, tag=which + "T", bufs=2,
                      name=f"{which}T{b}")
        for s in range(NSPAN):
            pp = ps.tile([128, SPAN], F32, tag="pps", bufs=2,
                         name=f"pp{which}{b}_{s}")
            for ch in range(DCH):
                nc.tensor.matmul(pp[:], w_sb[:, ch, :],
                                 xT[:, ch, 2 * s:2 * s + 2, :],
                                 start=(ch == 0), stop=(ch == DCH - 1))
            nc.vector.tensor_scalar_add(
                out=out[:, s * SPAN:(s + 1) * SPAN], in0=pp[:], scalar1=b_sb[:])
        st[(which + "T", b)] = out

    def proj_v(b):
        xT = st[("xT", b)]
        vsb = sb.tile([128, T // KT, 2 * (HD + 1)], BF16, tag="vsb", bufs=2,
                      name=f"vsb{b}")
        nc.vector.memset(vsb[:, :, HD:HD + 1], 1.0)
        nc.vector.memset(vsb[:, :, 2 * HD + 1:2 * HD + 2], 1.0)
        for g in range(T // KT // 4):
            psv = ps.tile([128, 4, 128], F32, tag="pps", bufs=2,
                          name=f"psv{b}_{g}")
            for u in range(4):
                tt = g * 4 + u
                for ch in range(DCH):
                    nc.tensor.matmul(
                        psv[:, u, :],
                        xT[:, ch, tt // 2,
                           (tt % 2) * 128:(tt % 2) * 128 + 128],
                        wv_sb[:, ch, :],
                        start=(ch == 0), stop=(ch == DCH - 1))
            nc.vector.tensor_copy(out=vsb[:, g * 4:(g + 1) * 4, 0:HD],
                                  in_=psv[:, :, 0:HD])
            nc.vector.tensor_copy(
                out=vsb[:, g * 4:(g + 1) * 4, HD + 1:2 * HD + 1],
                in_=psv[:, :, HD:2 * HD])
        st[("vsb", b)] = vsb

    def attn_pre(b):
        st[("osb", b)] = sb.tile([128, T], F32, tag="osb", bufs=1,
                                 name=f"osb{b}")
        st[("den0", b)] = sb.tile([1, T], F32, tag="den0", bufs=1,
                                  name=f"den0_{b}")
        st[("den1", b)] = sb.tile([1, T], F32, tag="den1", bufs=1,
                                  name=f"den1_{b}")

    def attn_group(b, j):
        qT, kT_, vsb = st[("qT", b)], st[("kT", b)], st[("vsb", b)]
        osb = st[("osb", b)]
        dens = (st[("den0", b)], st[("den1", b)])
        nk = (j + 1) * (SPAN // KT)
        opsh = [ps.tile([128, SPAN], F32, tag="ops", bufs=2,
                        name=f"ops{b}_{j}_{h}") for h in range(2)]
        pend = []

        def scores_step(i):
            dlt = i - j * (SPAN // KT)
            lo = KT * dlt if dlt >= 0 else 0
            sps = ps.tile([128, 2, SPAN], F32, tag="sps", bufs=2,
                          name=f"sps{b}_{j}_{i}")
            for h in range(2):
                hr = h * HD
                nc.tensor.matmul(
                    sps[:, h, lo:SPAN],
                    kT_[hr:hr + HD, i * KT:(i + 1) * KT],
                    qT[hr:hr + HD, j * SPAN + lo:(j + 1) * SPAN],
                    start=True, stop=True)
            et = sb.tile([128, 2, SPAN], BF16, tag="et", bufs=LOOKAHEAD + 2,
                         name=f"et{b}_{j}_{i}")
            nc.scalar.activation(out=et[:, :, lo:SPAN], in_=sps[:, :, lo:SPAN],
                                 func=AF.Exp, scale=0.125)
            if dlt >= 0:
                nc.vector.tensor_mul(out=et[:, :, lo:lo + KT],
                                     in0=et[:, :, lo:lo + KT], in1=mask2[:])
            return (et, lo)

        def pv_step(i, et, lo):
            for h in range(2):
                vcol = h * (HD + 1)
                nc.tensor.matmul(
                    opsh[h][0:HD + 1, lo:SPAN],
                    vsb[:, i, vcol:vcol + HD + 1], et[:, h, lo:SPAN],
                    start=(i == 0), stop=(i == nk - 1))

        for t in range(nk + LOOKAHEAD):
            if t < nk:
                pend.append((t,) + scores_step(t))
            if t >= LOOKAHEAD:
                pv_step(*pend.pop(0))
        while pend:
            pv_step(*pend.pop(0))
        for h in range(2):
            nc.vector.tensor_copy(
                out=osb[h * HD:(h + 1) * HD, j * SPAN:(j + 1) * SPAN],
                in_=opsh[h][0:HD, :])
            nc.vector.tensor_copy(
                out=dens[h][0:1, j * SPAN:(j + 1) * SPAN],
                in_=opsh[h][HD:HD + 1, :])

    def norm_a2a(b):
        osb = st[("osb", b)]
        rbcs = sb.tile([128, T], F32, tag="rbcs", bufs=1, name=f"rbcs{b}")
        onb = sb.tile([128, T], BF16, tag="onb", bufs=1, name=f"onb{b}")
        nc.gpsimd.partition_broadcast(rbcs[:], st[("den0", b)][0:1, :])
        nc.vector.reciprocal(out=rbcs[0:HD, :], in_=rbcs[0:HD, :])
        nc.vector.tensor_mul(out=onb[0:HD, :], in0=osb[0:HD, :],
                             in1=rbcs[0:HD, :])
        nc.gpsimd.partition_broadcast(rbcs[:], st[("den1", b)][0:1, :])
        nc.vector.reciprocal(out=rbcs[HD:128, :], in_=rbcs[HD:128, :])
        nc.vector.tensor_mul(out=onb[HD:128, :], in0=osb[HD:128, :],
                             in1=rbcs[HD:128, :])
        if debug and b == 0:
            nc.sync.dma_start(out=dbg["d_osb0"], in_=osb[:])
            nc.sync.dma_start(out=dbg["d_den0"], in_=st[("den0", b)][:])
            nc.sync.dma_start(out=dbg["d_den1"], in_=st[("den1", b)][:])
            nc.sync.dma_start(out=dbg["d_onb0"], in_=onb[:])
        a2a_in = dram.tile([NCORES, DHC, STRIP], BF16, name=f"a2a_in{b}")
        for r in range(NCORES):
            nc.sync.dma_start(out=a2a_in[r],
                              in_=onb[:, r * STRIP:(r + 1) * STRIP])
        a2a_out = dram.tile([NCORES, DHC, STRIP], BF16, name=f"a2a_out{b}")
        if no_cc:
            nc.gpsimd.dma_start(out=a2a_out[:], in_=a2a_in[:])
        else:
            nc.gpsimd.collective_compute(
                "AllToAll", mybir.AluOpType.bypass, replica_groups=RG,
                ins=[a2a_in.opt()], outs=[a2a_out.opt()])
        st[("a2a", b)] = a2a_out

    def outproj(b):
        og = sb.tile([128, NCORES, STRIP], BF16, tag="og", bufs=2,
                     name=f"og{b}")
        nc.sync.dma_start(out=og[:],
                          in_=st[("a2a", b)][:].rearrange("r p m -> p r m"))
        for t in range(STRIP // 128):
            for s in range(D // SPAN):
                yps = ps.tile([128, SPAN], F32, tag="pps", bufs=2,
                              name=f"yps{b}_{t}_{s}")
                for r in range(NCORES):
                    nc.tensor.matmul(yps[:], og[:, r, t * 128:(t + 1) * 128],
                                     wo_sb[:, r, s * SPAN:(s + 1) * SPAN],
                                     start=(r == 0), stop=False)
                nc.tensor.matmul(yps[:], ones_row[:],
                                 c_sb[0:1, s * SPAN:(s + 1) * SPAN],
                                 start=False, stop=True)
                ysb = sb.tile([128, SPAN], F32, tag="ysb", bufs=3,
                              name=f"ysb{b}_{t}_{s}")
                nc.vector.tensor_copy(out=ysb[:], in_=yps[:])
                nc.sync.dma_start(
                    out=y[b, t * 128:(t + 1) * 128, s * SPAN:(s + 1) * SPAN],
                    in_=ysb[:])

    # ---- schedule: batch-b attention interleaved with batch-(b+1) projs ----
    load_xT(0)
    proj_qk(0, "q")
    proj_qk(0, "k")
    proj_v(0)
    if debug:
        nc.sync.dma_start(out=dbg["d_qT0"], in_=st[("qT", 0)][:])
        nc.sync.dma_start(out=dbg["d_kT0"], in_=st[("kT", 0)][:])
        nc.sync.dma_start(out=dbg["d_vsb0"], in_=st[("vsb", 0)][:])
        nc.sync.dma_start(out=dbg["d_csb"], in_=c_sb[:])
    load_xT(1)
    for b in range(B):
        attn_pre(b)
        for j in range(NSPAN):
            attn_group(b, j)
            if j == 2 and b + 1 < B:
                proj_qk(b + 1, "q")
            if j == 3:
                if b + 1 < B:
                    proj_qk(b + 1, "k")
                if b >= 1:
                    outproj(b - 1)
        if b + 1 < B:
            proj_v(b + 1)
        norm_a2a(b)
        if b + 2 < B:
            load_xT(b + 2)
    outproj(B - 1)


def build(debug=False, tiny_out=False, reps=1, no_cc=False):
    nc = bacc.Bacc("TRN2", target_bir_lowering=False, debug=False,
                   num_devices=NCORES)
    dbg = {}
    if debug:
        for nm, shape in [("d_qT0", [128, T]), ("d_kT0", [128, T]),
                          ("d_vsb0", [128, T // KT, 2 * (HD + 1)]),
                          ("d_osb0", [128, T]), ("d_den0", [1, T]),
                          ("d_den1", [1, T]), ("d_onb0", [128, T]),
                          ("d_csb", [1, D])]:
            dt = BF16 if nm in ("d_qT0", "d_kT0", "d_vsb0", "d_onb0",
                                "d_csb") else F32
            dbg[nm] = nc.dram_tensor(nm, shape, dt, kind="ExternalOutput").ap()

    xs = nc.dram_tensor("xs", [B, STRIP, D], F32, kind="ExternalInput").ap()
    wq = nc.dram_tensor("wq", [D, DHC], F32, kind="ExternalInput").ap()
    wk = nc.dram_tensor("wk", [D, DHC], F32, kind="ExternalInput").ap()
    wv = nc.dram_tensor("wv", [D, DHC], F32, kind="ExternalInput").ap()
    wo = nc.dram_tensor("wo", [D, D], F32, kind="ExternalInput").ap()
    bq = nc.dram_tensor("bq", [DHC], F32, kind="ExternalInput").ap()
    bk = nc.dram_tensor("bk", [DHC], F32, kind="ExternalInput").ap()
    bv = nc.dram_tensor("bv", [D], F32, kind="ExternalInput").ap()
    bo = nc.dram_tensor("bo", [D], F32, kind="ExternalInput").ap()
    if tiny_out:
        y = nc.dram_tensor("y_int", [B, STRIP, D], F32, kind="Internal").ap()
        yd = nc.dram_tensor("yd", [128, 128], F32, kind="ExternalOutput").ap()
    else:
        y = nc.dram_tensor("y", [B, STRIP, D], F32, kind="ExternalOutput").ap()

    with tile.TileContext(nc) as tc:
        with tc.tile_pool(name="dram", bufs=1, space="DRAM") as dram, \
             tc.tile_pool(name="const", bufs=1) as const, \
             tc.tile_pool(name="sb", bufs=1) as sb, \
             tc.tile_pool(name="ps", bufs=1, space="PSUM") as ps:
            for _rep in range(reps):
                _emit(nc, dram, const, sb, ps, dbg,
                      xs, wq, wk, wv, wo, bq, bk, bv, bo, y,
                      debug=debug, no_cc=no_cc)
            if tiny_out:
                ydt = sb.tile([128, 128], F32, name="ydt")
                nc.vector.memset(ydt[:], 0.0)
                nc.sync.dma_start(out=yd, in_=ydt[:])
    nc.compile()
    return nc


_CACHE: dict = {}


def _get_nc():
    if "nc" not in _CACHE:
        _CACHE["nc"] = build()
    return _CACHE["nc"]


def make_in_maps(inputs):
    x = np.ascontiguousarray(np.asarray(inputs["x"], dtype=np.float32))
    wq = np.asarray(inputs["wq"], dtype=np.float32)
    wk = np.asarray(inputs["wk"], dtype=np.float32)
    wv = np.asarray(inputs["wv"], dtype=np.float32)
    wo = np.ascontiguousarray(np.asarray(inputs["wo"], dtype=np.float32))
    bq = np.asarray(inputs["bq"], dtype=np.float32)
    bk = np.asarray(inputs["bk"], dtype=np.float32)
    bv = np.ascontiguousarray(np.asarray(inputs["bv"], dtype=np.float32))
    bo = np.ascontiguousarray(np.asarray(inputs["bo"], dtype=np.float32))
    in_maps = []
    for c in range(NCORES):
        in_maps.append({
            "xs": np.ascontiguousarray(x[:, STRIP * c:STRIP * (c + 1), :]),
            "wq": np.ascontiguousarray(wq[:, DHC * c:DHC * (c + 1)]),
            "wk": np.ascontiguousarray(wk[:, DHC * c:DHC * (c + 1)]),
            "wv": np.ascontiguousarray(wv[:, DHC * c:DHC * (c + 1)]),
            "wo": wo,
            "bq": np.ascontiguousarray(bq[DHC * c:DHC * (c + 1)]),
            "bk": np.ascontiguousarray(bk[DHC * c:DHC * (c + 1)]),
            "bv": bv,
            "bo": bo,
        })
    return in_maps


def assemble_y(results):
    yf = np.empty((B, T, D), np.float32)
    for c in range(NCORES):
        yf[:, STRIP * c:STRIP * (c + 1), :] = results[c]["y"]
    return yf


def run(inputs, **kwargs):
    """Run on 8 cores; returns (y_full, BassKernelResults)."""
    res = bass_utils.run_bass_kernel_spmd(
        _get_nc(), make_in_maps(inputs), core_ids=list(range(NCORES)), **kwargs)
    return assemble_y(res.results), res


def kernel(**inputs) -> np.ndarray:
    yf, _ = run(inputs)
    return yf


# revision 19
# speedup vs baseline: 1.0692x; 1.0692x over previous
"""nn_MultiHeadAttention on 8 Trainium2 NeuronCores (Bass/Tile).

Reference computation (torch nn.MultiheadAttention, eval, causal):
    q = x @ wq + bq ; k = x @ wk + bk ; v = x @ wv + bv   (16 heads of 64)
    out = softmax(causal(q k^T / 8)) @ v ;  y = out @ wo + bo

Sharding (8 cores): head-parallel attention (2 heads/core, all batches),
then token-parallel output projection after per-batch AllToAlls.

Per-core pipeline:
  0. cast its token strips of x to bf16, per-batch AllGather -> full bf16 x.
  1. x^T via xbar transpose-DMA; project Q^T/K^T ([hd,tok] bf16) and V
     ([tok,hd] with a ones column -> the PV matmul also yields the softmax
     denominators).
  2. Causal attention in the S^T layout [k-tok, q-tok]: per (q-span, k-tile)
     both heads' scores land in one 2-bank PSUM pair (disjoint PE row
     groups), one exp on ACT per pair (scale=1/8 folded in; no
     max-subtraction needed: |s/8| < ~7), diagonal tiles compute only live
     columns plus one [128,128] bf16 boundary mask applied after exp.
     exp->PV is software-pipelined with a small lookahead.
  3. Normalize O^T by 1/denom (partition_broadcast + reciprocal), per-batch
     AllToAll, then the out-projection for this core's token strip of that
     batch (bias row via a K=1 matmul with c = bv@wo + bo, exact because
     softmax rows sum to 1).
Host: scatters the 8 [B, 256, D] chunks back into [B, T, D].
"""

import sys

if "/opt/trn_rl_repo" not in sys.path:
    sys.path.insert(0, "/opt/trn_rl_repo")

import numpy as np

import concourse.bass as bass  # noqa: F401
import concourse.mybir as mybir
import concourse.tile as tile
from concourse import bacc
from concourse import bass_utils

B, T, D, H, HD = 4, 2048, 1024, 16, 64
NCORES = 8
HPC = H // NCORES          # heads per core (2)
DHC = HPC * HD             # head dims per core (128)
STRIP = T // NCORES        # 256 tokens/batch contributed to AG & owned in y
SPAN = 512                 # q span (PSUM bank = 512 fp32)
KT = 128                   # k tile
NSPAN = T // SPAN          # 4
DCH = D // 128             # 8 contraction chunks
F32 = mybir.dt.float32
BF16 = mybir.dt.bfloat16
AF = mybir.ActivationFunctionType
RG = [list(range(NCORES))]
LOOKAHEAD = 4              # exp->PV software pipeline depth
J_DESC = True              # process attention q-span groups largest-first


def _emit(nc, dram, const, sb, ps, dbg, xs, wq, wk, wv, wo, bq, bk, bv, bo, y,
          *, debug, no_cc):
    # ---- constants / weights ----
    wq_sb = const.tile([128, DCH, DHC], BF16, tag="wq_sb", name="wq_sb")
    wk_sb = const.tile([128, DCH, DHC], BF16, tag="wk_sb", name="wk_sb")
    wv_sb = const.tile([128, DCH, DHC], BF16, tag="wv_sb", name="wv_sb")
    wo_sb = const.tile([128, DCH, D], BF16, tag="wo_sb", name="wo_sb")
    nc.gpsimd.dma_start(out=wq_sb[:], in_=wq.rearrange("(c p) m -> p c m", p=128))
    nc.gpsimd.dma_start(out=wk_sb[:], in_=wk.rearrange("(c p) m -> p c m", p=128))
    nc.gpsimd.dma_start(out=wv_sb[:], in_=wv.rearrange("(c p) m -> p c m", p=128))
    nc.gpsimd.dma_start(out=wo_sb[:], in_=wo.rearrange("(c p) m -> p c m", p=128))
    bq_sb = const.tile([128, 1], F32, tag="bq_sb", name="bq_sb")
    bk_sb = const.tile([128, 1], F32, tag="bk_sb", name="bk_sb")
    nc.sync.dma_start(out=bq_sb[:], in_=bq.rearrange("(p m) -> p m", m=1))
    nc.sync.dma_start(out=bk_sb[:], in_=bk.rearrange("(p m) -> p m", m=1))
    bv_sb = const.tile([128, DCH], BF16, tag="bv_sb", name="bv_sb")
    nc.gpsimd.dma_start(out=bv_sb[:], in_=bv.rearrange("(c p) -> p c", p=128))
    bo_sb = const.tile([1, D], F32, tag="bo_sb", name="bo_sb")
    nc.sync.dma_start(out=bo_sb[:], in_=bo.rearrange("(o m) -> o m", o=1))

    ones_row = const.tile([1, 128], BF16, tag="ones_row", name="ones_row")
    nc.vector.memset(ones_row[:], 1.0)
    # boundary mask (multiplicative, post-exp): keep iff qoff >= p,
    # duplicated for the two head halves of an et pair tile.
    mask2 = const.tile([128, 2, KT], BF16, tag="mask2", name="mask2")
    nc.gpsimd.memset(mask2[:], 1.0)
    for u in range(2):
        nc.gpsimd.affine_select(
            out=mask2[:, u, :], in_=mask2[:, u, :],
            pattern=[[1, KT]], compare_op=mybir.AluOpType.is_ge,
            fill=0.0, base=0, channel_multiplier=-1)

    ident = const.tile([128, 128], BF16, tag="ident", name="ident")
    from concourse.masks import make_identity
    make_identity(nc, ident[:])

    # c = bv @ wo + bo (exact bias row for y)
    c_sb = const.tile([1, D], BF16, tag="c_sb", name="c_sb")
    for s in range(D // SPAN):
        cps = ps.tile([128, SPAN], F32, tag="pps", bufs=2, name=f"cps{s}")
        for ch in range(DCH):
            nc.tensor.matmul(cps[0:1, :], bv_sb[:, ch:ch + 1],
                             wo_sb[:, ch, s * SPAN:(s + 1) * SPAN],
                             start=(ch == 0), stop=(ch == DCH - 1))
        nc.vector.tensor_add(out=c_sb[0:1, s * SPAN:(s + 1) * SPAN],
                             in0=cps[0:1, :],
                             in1=bo_sb[0:1, s * SPAN:(s + 1) * SPAN])

    # ---- bf16 cast, PE-transpose own strip, AllGather of x^T (per batch) ----
    xg = []
    for b in range(B):
        xst_sb = sb.tile([128, 2, D], BF16, tag="xst", bufs=1, name=f"xst{b}")
        nc.gpsimd.dma_start(out=xst_sb[:],
                            in_=xs[b].rearrange("(g p) d -> p g d", p=128))
        xts = sb.tile([128, DCH, STRIP], BF16, tag="xts", bufs=2,
                      name=f"xts{b}")
        for g in range(2):
            for ch in range(DCH):
                tp = ps.tile([128, 128], BF16, tag="pps", bufs=2,
                             name=f"tp{b}_{g}_{ch}")
                nc.tensor.transpose(tp[:], xst_sb[:, g, ch * 128:(ch + 1) * 128],
                                    ident[:])
                nc.vector.tensor_copy(out=xts[:, ch, g * 128:(g + 1) * 128],
                                      in_=tp[:])
        xtd = dram.tile([D, STRIP], BF16, name=f"xtd{b}")
        nc.sync.dma_start(out=xtd.opt().rearrange("(c p) m -> p c m", p=128),
                          in_=xts[:])
        xgb = dram.tile([NCORES, D, STRIP], BF16, name=f"xg{b}",
                        addr_space="Local" if no_cc else "Shared")
        if no_cc:
            for _r in range(NCORES):
                nc.gpsimd.dma_start(out=xgb[_r], in_=xtd[:])
        else:
            nc.gpsimd.collective_compute(
                "AllGather", mybir.AluOpType.bypass, replica_groups=RG,
                ins=[xtd.opt()], outs=[xgb.opt()])
        xg.append(xgb)

    st = {}  # per-batch tiles

    def load_xT(b):
        # [128, ch, g, m]: d = 128*ch + p, token = 256*g + m
        xT = sb.tile([128, DCH, NCORES, STRIP], BF16, tag="xT", bufs=1,
                     name=f"xT{b}")
        for g in range(NCORES):
            nc.sync.dma_start(
                out=xT[:, :, g, :],
                in_=xg[b][g].rearrange("(c p) m -> p c m", p=128))
        st[("xT", b)] = xT

    def proj_qk(b, which):
        xT = st[("xT", b)]
        w_sb, b_sb = (wq_sb, bq_sb) if which == "q" else (wk_sb, bk_sb)
        out = sb.tile([128, T], BF16, tag=which + "T", bufs=2,
                      name=f"{which}T{b}")
        for s in range(NSPAN):
            pp = ps.tile([128, SPAN], F32, tag="pps", bufs=2,
                         name=f"pp{which}{b}_{s}")
            for ch in range(DCH):
                nc.tensor.matmul(pp[:], w_sb[:, ch, :],
                                 xT[:, ch, 2 * s:2 * s + 2, :],
                                 start=(ch == 0), stop=(ch == DCH - 1))
            nc.vector.tensor_scalar_add(
                out=out[:, s * SPAN:(s + 1) * SPAN], in0=pp[:], scalar1=b_sb[:])
        st[(which + "T", b)] = out

    def proj_v(b):
        xT = st[("xT", b)]
        vsb = sb.tile([128, T // KT, 2 * (HD + 1)], BF16, tag="vsb", bufs=2,
                      name=f"vsb{b}")
        nc.vector.memset(vsb[:, :, HD:HD + 1], 1.0)
        nc.vector.memset(vsb[:, :, 2 * HD + 1:2 * HD + 2], 1.0)
        for g in range(T // KT // 4):
            psv = ps.tile([128, 4, 128], F32, tag="pps", bufs=2,
                          name=f"psv{b}_{g}")
            for u in range(4):
                tt = g * 4 + u
                for ch in range(DCH):
                    nc.tensor.matmul(
                        psv[:, u, :],
                        xT[:, ch, tt // 2,
                           (tt % 2) * 128:(tt % 2) * 128 + 128],
                        wv_sb[:, ch, :],
                        start=(ch == 0), stop=(ch == DCH - 1))
            nc.vector.tensor_copy(out=vsb[:, g * 4:(g + 1) * 4, 0:HD],
                                  in_=psv[:, :, 0:HD])
            nc.vector.tensor_copy(
                out=vsb[:, g * 4:(g + 1) * 4, HD + 1:2 * HD + 1],
                in_=psv[:, :, HD:2 * HD])
        st[("vsb", b)] = vsb

    def attn_pre(b):
        st[("osb", b)] = sb.tile([128, T], F32, tag="osb", bufs=1,
                                 name=f"osb{b}")
        st[("den0", b)] = sb.tile([1, T], F32, tag="den0", bufs=1,
                                  name=f"den0_{b}")
        st[("den1", b)] = sb.tile([1, T], F32, tag="den1", bufs=1,
                                  name=f"den1_{b}")

    def attn_group(b, j):
        qT, kT_, vsb = st[("qT", b)], st[("kT", b)], st[("vsb", b)]
        osb = st[("osb", b)]
        dens = (st[("den0", b)], st[("den1", b)])
        nk = (j + 1) * (SPAN // KT)
        opsh = [ps.tile([128, SPAN], F32, tag="ops", bufs=2,
                        name=f"ops{b}_{j}_{h}") for h in range(2)]
        pend = []

        def scores_step(i):
            dlt = i - j * (SPAN // KT)
            lo = KT * dlt if dlt >= 0 else 0
            sps = ps.tile([128, 2, SPAN], F32, tag="sps", bufs=2,
                          name=f"sps{b}_{j}_{i}")
            for h in range(2):
                hr = h * HD
                nc.tensor.matmul(
                    sps[:, h, lo:SPAN],
                    kT_[hr:hr + HD, i * KT:(i + 1) * KT],
                    qT[hr:hr + HD, j * SPAN + lo:(j + 1) * SPAN],
                    start=True, stop=True)
            et = sb.tile([128, 2, SPAN], BF16, tag="et", bufs=LOOKAHEAD + 2,
                         name=f"et{b}_{j}_{i}")
            nc.scalar.activation(out=et[:, :, lo:SPAN], in_=sps[:, :, lo:SPAN],
                                 func=AF.Exp, scale=0.125)
            if dlt >= 0:
                nc.vector.tensor_mul(out=et[:, :, lo:lo + KT],
                                     in0=et[:, :, lo:lo + KT], in1=mask2[:])
            return (et, lo)

        def pv_step(i, et, lo):
            for h in range(2):
                vcol = h * (HD + 1)
                nc.tensor.matmul(
                    opsh[h][0:HD + 1, lo:SPAN],
                    vsb[:, i, vcol:vcol + HD + 1], et[:, h, lo:SPAN],
                    start=(i == 0), stop=(i == nk - 1))

        for t in range(nk + LOOKAHEAD):
            if t < nk:
                pend.append((t,) + scores_step(t))
            if t >= LOOKAHEAD:
                pv_step(*pend.pop(0))
        while pend:
            pv_step(*pend.pop(0))
        for h in range(2):
            nc.vector.tensor_copy(
                out=osb[h * HD:(h + 1) * HD, j * SPAN:(j + 1) * SPAN],
                in_=opsh[h][0:HD, :])
            nc.vector.tensor_copy(
                out=dens[h][0:1, j * SPAN:(j + 1) * SPAN],
                in_=opsh[h][HD:HD + 1, :])

    def norm_a2a(b):
        osb = st[("osb", b)]
        rbcs = sb.tile([128, T], F32, tag="rbcs", bufs=1, name=f"rbcs{b}")
        onb = sb.tile([128, T], BF16, tag="onb", bufs=1, name=f"onb{b}")
        nc.gpsimd.partition_broadcast(rbcs[:], st[("den0", b)][0:1, :])
        nc.vector.reciprocal(out=rbcs[0:HD, :], in_=rbcs[0:HD, :])
        nc.vector.tensor_mul(out=onb[0:HD, :], in0=osb[0:HD, :],
                             in1=rbcs[0:HD, :])
        nc.gpsimd.partition_broadcast(rbcs[:], st[("den1", b)][0:1, :])
        nc.vector.reciprocal(out=rbcs[HD:128, :], in_=rbcs[HD:128, :])
        nc.vector.tensor_mul(out=onb[HD:128, :], in0=osb[HD:128, :],
                             in1=rbcs[HD:128, :])
        if debug and b == 0:
            nc.sync.dma_start(out=dbg["d_osb0"], in_=osb[:])
            nc.sync.dma_start(out=dbg["d_den0"], in_=st[("den0", b)][:])
            nc.sync.dma_start(out=dbg["d_den1"], in_=st[("den1", b)][:])
            nc.sync.dma_start(out=dbg["d_onb0"], in_=onb[:])
        a2a_in = dram.tile([NCORES, DHC, STRIP], BF16, name=f"a2a_in{b}")
        for r in range(NCORES):
            nc.sync.dma_start(out=a2a_in[r],
                              in_=onb[:, r * STRIP:(r + 1) * STRIP])
        a2a_out = dram.tile([NCORES, DHC, STRIP], BF16, name=f"a2a_out{b}")
        if no_cc:
            nc.gpsimd.dma_start(out=a2a_out[:], in_=a2a_in[:])
        else:
            nc.gpsimd.collective_compute(
                "AllToAll", mybir.AluOpType.bypass, replica_groups=RG,
                ins=[a2a_in.opt()], outs=[a2a_out.opt()])
        st[("a2a", b)] = a2a_out

    def outproj(b):
        og = sb.tile([128, NCORES, STRIP], BF16, tag="og", bufs=2,
                     name=f"og{b}")
        nc.sync.dma_start(out=og[:],
                          in_=st[("a2a", b)][:].rearrange("r p m -> p r m"))
        for t in range(STRIP // 128):
            for s in range(D // SPAN):
                yps = ps.tile([128, SPAN], F32, tag="pps", bufs=2,
                              name=f"yps{b}_{t}_{s}")
                for r in range(NCORES):
                    nc.tensor.matmul(yps[:], og[:, r, t * 128:(t + 1) * 128],
                                     wo_sb[:, r, s * SPAN:(s + 1) * SPAN],
                                     start=(r == 0), stop=False)
                nc.tensor.matmul(yps[:], ones_row[:],
                                 c_sb[0:1, s * SPAN:(s + 1) * SPAN],
                                 start=False, stop=True)
                ysb = sb.tile([128, SPAN], F32, tag="ysb", bufs=3,
                              name=f"ysb{b}_{t}_{s}")
                nc.vector.tensor_copy(out=ysb[:], in_=yps[:])
                nc.sync.dma_start(
                    out=y[b, t * 128:(t + 1) * 128, s * SPAN:(s + 1) * SPAN],
                    in_=ysb[:])

    # ---- schedule: batch-b attention interleaved with batch-(b+1) projs ----
    load_xT(0)
    proj_qk(0, "q")
    proj_qk(0, "k")
    proj_v(0)
    if debug:
        nc.sync.dma_start(out=dbg["d_qT0"], in_=st[("qT", 0)][:])
        nc.sync.dma_start(out=dbg["d_kT0"], in_=st[("kT", 0)][:])
        nc.sync.dma_start(out=dbg["d_vsb0"], in_=st[("vsb", 0)][:])
        nc.sync.dma_start(out=dbg["d_csb"], in_=c_sb[:])
    load_xT(1)
    for b in range(B):
        attn_pre(b)
        jorder = (3, 2, 1, 0) if J_DESC else (0, 1, 2, 3)
        for pos, j in enumerate(jorder):
            attn_group(b, j)
            if pos == 2 and b + 1 < B:
                proj_qk(b + 1, "q")
            if pos == 3:
                if b + 1 < B:
                    proj_qk(b + 1, "k")
                if b >= 1:
                    outproj(b - 1)
        if b + 1 < B:
            proj_v(b + 1)
        norm_a2a(b)
        if b + 2 < B:
            load_xT(b + 2)
    outproj(B - 1)


def build(debug=False, tiny_out=False, reps=1, no_cc=False):
    nc = bacc.Bacc("TRN2", target_bir_lowering=False, debug=False,
                   num_devices=NCORES)
    dbg = {}
    if debug:
        for nm, shape in [("d_qT0", [128, T]), ("d_kT0", [128, T]),
                          ("d_vsb0", [128, T // KT, 2 * (HD + 1)]),
                          ("d_osb0", [128, T]), ("d_den0", [1, T]),
                          ("d_den1", [1, T]), ("d_onb0", [128, T]),
                          ("d_csb", [1, D])]:
            dt = BF16 if nm in ("d_qT0", "d_kT0", "d_vsb0", "d_onb0",
                                "d_csb") else F32
            dbg[nm] = nc.dram_tensor(nm, shape, dt, kind="ExternalOutput").ap()

    xs = nc.dram_tensor("xs", [B, STRIP, D], F32, kind="ExternalInput").ap()
    wq = nc.dram_tensor("wq", [D, DHC], F32, kind="ExternalInput").ap()
    wk = nc.dram_tensor("wk", [D, DHC], F32, kind="ExternalInput").ap()
    wv = nc.dram_tensor("wv", [D, DHC], F32, kind="ExternalInput").ap()
    wo = nc.dram_tensor("wo", [D, D], F32, kind="ExternalInput").ap()
    bq = nc.dram_tensor("bq", [DHC], F32, kind="ExternalInput").ap()
    bk = nc.dram_tensor("bk", [DHC], F32, kind="ExternalInput").ap()
    bv = nc.dram_tensor("bv", [D], F32, kind="ExternalInput").ap()
    bo = nc.dram_tensor("bo", [D], F32, kind="ExternalInput").ap()
    if tiny_out:
        y = nc.dram_tensor("y_int", [B, STRIP, D], F32, kind="Internal").ap()
        yd = nc.dram_tensor("yd", [128, 128], F32, kind="ExternalOutput").ap()
    else:
        y = nc.dram_tensor("y", [B, STRIP, D], F32, kind="ExternalOutput").ap()

    with tile.TileContext(nc) as tc:
        with tc.tile_pool(name="dram", bufs=1, space="DRAM") as dram, \
             tc.tile_pool(name="const", bufs=1) as const, \
             tc.tile_pool(name="sb", bufs=1) as sb, \
             tc.tile_pool(name="ps", bufs=1, space="PSUM") as ps:
            for _rep in range(reps):
                _emit(nc, dram, const, sb, ps, dbg,
                      xs, wq, wk, wv, wo, bq, bk, bv, bo, y,
                      debug=debug, no_cc=no_cc)
            if tiny_out:
                ydt = sb.tile([128, 128], F32, name="ydt")
                nc.vector.memset(ydt[:], 0.0)
                nc.sync.dma_start(out=yd, in_=ydt[:])
    nc.compile()
    return nc


_CACHE: dict = {}


def _get_nc():
    if "nc" not in _CACHE:
        _CACHE["nc"] = build()
    return _CACHE["nc"]


def make_in_maps(inputs):
    x = np.ascontiguousarray(np.asarray(inputs["x"], dtype=np.float32))
    wq = np.asarray(inputs["wq"], dtype=np.float32)
    wk = np.asarray(inputs["wk"], dtype=np.float32)
    wv = np.asarray(inputs["wv"], dtype=np.float32)
    wo = np.ascontiguousarray(np.asarray(inputs["wo"], dtype=np.float32))
    bq = np.asarray(inputs["bq"], dtype=np.float32)
    bk = np.asarray(inputs["bk"], dtype=np.float32)
    bv = np.ascontiguousarray(np.asarray(inputs["bv"], dtype=np.float32))
    bo = np.ascontiguousarray(np.asarray(inputs["bo"], dtype=np.float32))
    in_maps = []
    for c in range(NCORES):
        in_maps.append({
            "xs": np.ascontiguousarray(x[:, STRIP * c:STRIP * (c + 1), :]),
            "wq": np.ascontiguousarray(wq[:, DHC * c:DHC * (c + 1)]),
            "wk": np.ascontiguousarray(wk[:, DHC * c:DHC * (c + 1)]),
            "wv": np.ascontiguousarray(wv[:, DHC * c:DHC * (c + 1)]),
            "wo": wo,
            "bq": np.ascontiguousarray(bq[DHC * c:DHC * (c + 1)]),
            "bk": np.ascontiguousarray(bk[DHC * c:DHC * (c + 1)]),
            "bv": bv,
            "bo": bo,
        })
    return in_maps


def assemble_y(results):
    yf = np.empty((B, T, D), np.float32)
    for c in range(NCORES):
        yf[:, STRIP * c:STRIP * (c + 1), :] = results[c]["y"]
    return yf


def run(inputs, **kwargs):
    """Run on 8 cores; returns (y_full, BassKernelResults)."""
    res = bass_utils.run_bass_kernel_spmd(
        _get_nc(), make_in_maps(inputs), core_ids=list(range(NCORES)), **kwargs)
    return assemble_y(res.results), res


def kernel(**inputs) -> np.ndarray:
    yf, _ = run(inputs)
    return yf
